# revision 17
# baseline (speedup 1.0000x reference)
# Trainium2 Bass kernel for nn_Decoder (LSTM decoder + GCN message passing).
#
# Strategy (8 NeuronCores, SPMD), v3:
#   * Nodes padded N=10000 -> 10240; each core owns 1280 nodes.
#   * LSTM is feature-major ([H, nodes]) in 3 chunks (512/512/256) per step,
#     gate-major PSUM layout [i|f|o|g] so each gate region is one PSUM bank
#     (one 2-matmul accumulation group per bank).  ALL FOUR gates go through
#     a single sigmoid ACT call per chunk: tanh(g) = 2*sigmoid(2g)-1 with the
#     x2 folded into the g-gate weights host-side and the affine fixup done
#     on GpSimd.  tanh(c) is batched per chunk-pair.  Elementwise work is
#     split across DVE and GpSimd so ACT (the bottleneck) stays ~saturated.
#   * Y projection (h @ (W_gcn@W_fc3)) for step t is issued early in step
#     t+1 into the then-dead g-gate PSUM banks, keeping PE dense; the
#     mask*dinv[src] scaling is folded into the PSUM->SBUF evacuation
#     multiply against a host-precomputed replicated scale table.
#   * Local Y table ysb is [p, t, k, f] so time slices are contiguous;
#     slices (t=0:6, 6:12) are shipped via AllGather (first at step 6 so it
#     overlaps the LSTM tail); a tiny dummy collective at t~0 absorbs the
#     rendezvous barrier, with its bounce DMA on the gpsimd queue so the
#     critical input loads on the sync queue are not blocked behind it.
#   * GCN scatter = block-dense matmul agg[dst] = sum_sb A[sb,dst].T @ Y[sb]
#     with A in fp8e4 (exact small ints), prefetched whole into SBUF during
#     the LSTM; 800 matmuls of 192 cols run warm at the PE issue roofline.
import os
import numpy as np
import ml_dtypes

import concourse.bass as bass
import concourse.bacc as bacc
import concourse.tile as tile
from concourse import mybir
from concourse import bass_utils

P = 128
N, T, NF, H, L, E = 10000, 12, 16, 128, 64, 160000
NCORES = 8
NCN_RAW = N // NCORES        # 1250 real nodes per core
NCN = 1280                   # padded nodes per core (10 full tiles)
NP = NCN * NCORES            # 10240 padded global nodes
NT = NCN // P                # 10 dst tiles per core
NSB = NP // P                # 80 source blocks
TNF = T * NF                 # 192
SLICES = [(0, 7), (7, 12)]   # t-ranges of the shipped Y slices

F32 = mybir.dt.float32
F16 = mybir.dt.float16
F8 = mybir.dt.float8e4

_BUILD_CACHE = {}
LAST_RESULTS = None  # BassKernelResults of the most recent run (for test harness)


def _build():
    nc = bacc.Bacc("TRN2", target_bir_lowering=False, debug=False,
                   num_devices=NCORES)

    # ---------------- I/O declarations ----------------
    zT = nc.dram_tensor("zT", [L + 1, NCN], F16, kind="ExternalInput")
    wzg = nc.dram_tensor("wzg", [L + 1, 4 * H], F16, kind="ExternalInput")
    wfc2 = nc.dram_tensor("wfc2", [L, H], F16, kind="ExternalInput")
    b2 = nc.dram_tensor("b2", [P, 1], F32, kind="ExternalInput")
    whh = nc.dram_tensor("whh", [H, 4 * H], F16, kind="ExternalInput")
    wcomb = nc.dram_tensor("wcomb", [H, NF], F16, kind="ExternalInput")
    mdvrep = nc.dram_tensor("mdvrep", [P, NT * NF], F32, kind="ExternalInput")
    dinvt = nc.dram_tensor("dinvt", [P, NT], F32, kind="ExternalInput")
    bout = nc.dram_tensor("bout", [P, TNF], F32, kind="ExternalInput")
    # A blocks: ablk[p, (sb*NT + kt)*P + d] = multiplicity of edge
    # (src = sb*128+p, dst_local = kt*128+d), fp8 (exact small ints).
    ablk = nc.dram_tensor("ablk", [P, NSB * NT * P], F8, kind="ExternalInput")
    xhat = nc.dram_tensor("xhat", [NCN, TNF], F32, kind="ExternalOutput")

    SIG = mybir.ActivationFunctionType.Sigmoid
    TANH = mybir.ActivationFunctionType.Tanh
    MUL = mybir.AluOpType.mult
    ADD = mybir.AluOpType.add

    with tile.TileContext(nc) as tc:
        with tc.tile_pool(name="cpool", bufs=1) as cp, \
             tc.tile_pool(name="spool", bufs=1) as sp, \
             tc.tile_pool(name="dram", bufs=1, space="DRAM") as dp:

            # ---- tiny dummy collective: absorb the rendezvous barrier.
            # Its bounce DMA goes on the gpsimd queue so the sync-queue input
            # loads are not serialized behind the barrier.
            db_i = dp.tile([P, 1], F32, name="db_i")
            db_o = dp.tile([NCORES * P, 1], F32, addr_space="Shared",
                           name="db_o")
            dbs = cp.tile([P, 1], F32)
            nc.gpsimd.memset(dbs[:], 0.0)
            nc.gpsimd.dma_start(db_i[:], dbs[:])
            nc.gpsimd.collective_compute(
                "AllGather", mybir.AluOpType.bypass,
                replica_groups=[list(range(NCORES))],
                ins=[db_i.opt()], outs=[db_o.opt()])

            # ---- constant loads (critical ones first) ----
            zt_sb = cp.tile([L + 1, NCN], F16)
            nc.sync.dma_start(zt_sb[:], zT[:])
            wfc2_sb = cp.tile([L, H], F16)
            nc.sync.dma_start(wfc2_sb[:], wfc2[:])
            wzg_sb = cp.tile([L + 1, 4 * H], F16)
            nc.sync.dma_start(wzg_sb[:], wzg[:])
            b2_sb = cp.tile([P, 1], F32)
            nc.sync.dma_start(b2_sb[:], b2[:])
            whh_sb = cp.tile([H, 4 * H], F16)
            nc.sync.dma_start(whh_sb[:], whh[:])
            wcomb_sb = cp.tile([H, NF], F16)
            nc.sync.dma_start(wcomb_sb[:], wcomb[:])
            mdv_sb = cp.tile([P, NT * NF], F32)
            nc.sync.dma_start(mdv_sb[:], mdvrep[:])
            dinv_sb = cp.tile([P, NT], F32)
            nc.sync.dma_start(dinv_sb[:], dinvt[:])
            bout_sb = cp.tile([P, TNF], F32)
            nc.sync.dma_start(bout_sb[:], bout[:])

            # ---- A prefetch target + gate ----
            asb = sp.tile([P, NSB * NT * P], F8, name="asb")
            pfgate = cp.tile([P, 1], F16)

            # ---- state tensors ----
            hdT = sp.tile([H, NCN], F16)       # fc2 output == h0
            hT0 = sp.tile([H, NCN], F16)       # h_t (even steps)
            hT1 = sp.tile([H, NCN], F16)       # h_t (odd steps)
            cT = sp.tile([H, NCN], F16)        # c_t
            thc = sp.tile([H, NCN], F16)       # tanh(c_t)
            tmp = sp.tile([H, NCN], F16)       # i * g'
            nc.vector.memset(cT[:], 0.0)
            sgA = sp.tile([P, 2048], F16, name="sgA")
            sgB = sp.tile([P, 2048], F16, name="sgB")
            sgC = sp.tile([P, 1024], F16, name="sgC")
            ysb = sp.tile([P, T * NT * NF], F16, name="ysb")   # [p, t, k, f]
            ysb_r = ysb[:].rearrange("p (t k f) -> p t k f", t=T, f=NF)
            # each shipped t-range lands in its own contiguous region:
            # cols [0 : 80*112) hold t=0:7, cols [80*112 :) hold t=7:12
            ytab = sp.tile([P, NSB * TNF], F16, name="ytab")
            HB = NSB * 7 * NF
            ytab_h = [ytab[:, 0:HB].rearrange("p (sb t f) -> p sb t f",
                                              t=7, f=NF),
                      ytab[:, HB:].rearrange("p (sb t f) -> p sb t f",
                                             t=5, f=NF)]

            ysh = [dp.tile([NCN, (t1 - t0) * NF], F16, name=f"ysh{i}")
                   for i, (t0, t1) in enumerate(SLICES)]
            yfull = [dp.tile([NP, (t1 - t0) * NF], F16,
                             addr_space="Shared", name=f"yfull{i}")
                     for i, (t0, t1) in enumerate(SLICES)]

            def ship_slice(si):
                t0, t1 = SLICES[si]
                nc.sync.dma_start(
                    ysh[si][:].rearrange("(k p) (t f) -> p t k f",
                                         p=P, f=NF),
                    ysb_r[:, t0:t1, :, :])
                nc.gpsimd.collective_compute(
                    "AllGather", mybir.AluOpType.bypass,
                    replica_groups=[list(range(NCORES))],
                    ins=[ysh[si].opt()], outs=[yfull[si].opt()])
                yf_r = yfull[si][:].rearrange("(sb p) (t f) -> p sb t f",
                                              p=P, f=NF)
                if si == 0:
                    # mid-LSTM: keep the assemble off the gpsimd engine
                    # queue -- its issue instruction would block the queue
                    # (and the LSTM's elementwise ops) until the AllGather
                    # completes.  The sync queue has no compute to stall.
                    nc.sync.dma_start(ytab_h[si][:, :, :, :], yf_r)
                else:
                    # after the LSTM: gpsimd is idle, split for bandwidth
                    nc.sync.dma_start(ytab_h[si][:, 0:NSB // 2, :, :],
                                      yf_r[:, 0:NSB // 2, :, :])
                    nc.gpsimd.dma_start(ytab_h[si][:, NSB // 2:, :, :],
                                        yf_r[:, NSB // 2:, :, :])

            with tc.tile_pool(name="psL", bufs=1, space="PSUM") as psL:
                S0 = psL.tile([P, 2048], F32, name="S0")
                S1 = psL.tile([P, 2048], F32, name="S1")

                # PE warmup burst: flip the HAM clock gate to 8/8 before the
                # LSTM starts; overlaps the remaining input DMAs.
                for _ in range(36):
                    nc.tensor.matmul(out=S1[:, 0:128], lhsT=wfc2_sb[:],
                                     rhs=zt_sb[:L, 0:128],
                                     start=True, stop=True)

                # ---- hd = z @ W_fc2 + b_fc2 (feature-major) ----
                for off, cw, S, po in ((0, 512, S0, 0), (512, 512, S1, 0),
                                       (1024, 256, S0, 512)):
                    nc.tensor.matmul(out=S[:, po:po + cw], lhsT=wfc2_sb[:],
                                     rhs=zt_sb[:L, off:off + cw],
                                     start=True, stop=True)
                    nc.vector.tensor_scalar(
                        out=hdT[:, off:off + cw], in0=S[:, po:po + cw],
                        scalar1=b2_sb[:, 0:1], scalar2=None, op0=ADD)

                # A prefetch, gated behind hdT so the critical input loads
                # get the DMA engines to themselves at startup.
                nc.gpsimd.tensor_copy(out=pfgate[:], in_=hdT[:, 0:1])
                APF = 4
                acols = NSB * NT * P
                for i in range(APF):
                    c0 = acols * i // APF
                    c1 = acols * (i + 1) // APF
                    nc.gpsimd.dma_start(asb[:, c0:c1], ablk[:, c0:c1])

                def gates_z(S, off, cw, gates=(0, 1, 2, 3)):
                    # z-side gate matmuls: depend only on zt, so they run at
                    # step start while the previous step's h is finishing.
                    for g in gates:
                        st = True if cw == 512 else (g % 2 == 0)
                        nc.tensor.matmul(
                            out=S[:, g * cw:(g + 1) * cw],
                            lhsT=wzg_sb[:, g * H:(g + 1) * H],
                            rhs=zt_sb[:, off:off + cw],
                            start=st, stop=False)

                def gates_h(S, off, cw, prev):
                    for g in range(4):
                        sp_ = True if cw == 512 else (g % 2 == 1)
                        nc.tensor.matmul(
                            out=S[:, g * cw:(g + 1) * cw],
                            lhsT=whh_sb[:, g * H:(g + 1) * H],
                            rhs=prev[:, off:off + cw],
                            start=False, stop=sp_)

                def gates_mm(S, off, cw, prev):
                    for g in range(4):
                        st = True if cw == 512 else (g % 2 == 0)
                        sp_ = True if cw == 512 else (g % 2 == 1)
                        nc.tensor.matmul(
                            out=S[:, g * cw:(g + 1) * cw],
                            lhsT=wzg_sb[:, g * H:(g + 1) * H],
                            rhs=zt_sb[:, off:off + cw],
                            start=st, stop=False)
                        nc.tensor.matmul(
                            out=S[:, g * cw:(g + 1) * cw],
                            lhsT=whh_sb[:, g * H:(g + 1) * H],
                            rhs=prev[:, off:off + cw],
                            start=False, stop=sp_)

                def yproj_chunk(pt, S, pbase, off, ntl, hsrc):
                    # Y projection for step pt: ntl node tiles starting at
                    # node offset `off`, into dead g-bank cols of S at
                    # `pbase`; evacuate with mask*dinv scale folded in.
                    k0 = off // P
                    for j in range(ntl):
                        nc.tensor.matmul(
                            out=S[:, pbase + j * NF:pbase + (j + 1) * NF],
                            lhsT=hsrc[:, off + j * P:off + (j + 1) * P],
                            rhs=wcomb_sb[:],
                            start=(j == 0), stop=(j == ntl - 1))
                    yb = (pt * NT + k0) * NF
                    nc.vector.tensor_mul(
                        out=ysb[:, yb:yb + ntl * NF],
                        in0=S[:, pbase:pbase + ntl * NF],
                        in1=mdv_sb[:, k0 * NF:(k0 + ntl) * NF])

                # ---- LSTM: T steps.  Step t opens with the previous step's
                # Y projections (reading the double-buffered h) plus all
                # z-side gate matmuls, none of which need h(t-1) -- so the PE
                # chews ~5us of work while DVE finishes the h products. ----
                for t in range(T):
                    hprev = hdT if t == 0 else (hT1 if (t - 1) % 2 else hT0)
                    hcur = hT1 if t % 2 else hT0
                    gates_z(S0, 0, 512, (0, 1, 2))
                    gates_z(S1, 512, 512)
                    if t > 0:
                        yproj_chunk(t - 1, S0, 1536, 0, 4, hprev)
                        yproj_chunk(t - 1, S0, 1600, 512, 4, hprev)
                        yproj_chunk(t - 1, S0, 1664, 1024, 2, hprev)
                    gates_z(S0, 0, 512, (3,))
                    gates_h(S0, 0, 512, hprev)
                    nc.scalar.activation(out=sgA[:, 0:2048],
                                         in_=S0[:, 0:2048], func=SIG)
                    gates_h(S1, 512, 512, hprev)
                    # elementwise for A (c = sig(f)*c + sig(i)*g')
                    nc.vector.tensor_scalar(
                        out=sgA[:, 1536:2048], in0=sgA[:, 1536:2048],
                        scalar1=2.0, scalar2=-1.0, op0=MUL, op1=ADD)
                    nc.vector.tensor_mul(out=cT[:, 0:512], in0=cT[:, 0:512],
                                         in1=sgA[:, 512:1024])
                    nc.gpsimd.tensor_mul(out=tmp[:, 0:512],
                                         in0=sgA[:, 0:512],
                                         in1=sgA[:, 1536:2048])
                    nc.vector.tensor_add(out=cT[:, 0:512], in0=cT[:, 0:512],
                                         in1=tmp[:, 0:512])
                    nc.scalar.activation(out=sgB[:, 0:2048],
                                         in_=S1[:, 0:2048], func=SIG)
                    # --- chunk C (nodes 1024:1280) gates, reuses S0[0:1024]
                    gates_mm(S0, 1024, 256, hprev)
                    # elementwise for B
                    nc.vector.tensor_scalar(
                        out=sgB[:, 1536:2048], in0=sgB[:, 1536:2048],
                        scalar1=2.0, scalar2=-1.0, op0=MUL, op1=ADD)
                    nc.vector.tensor_mul(out=cT[:, 512:1024],
                                         in0=cT[:, 512:1024],
                                         in1=sgB[:, 512:1024])
                    nc.gpsimd.tensor_mul(out=tmp[:, 512:1024],
                                         in0=sgB[:, 0:512],
                                         in1=sgB[:, 1536:2048])
                    nc.vector.tensor_add(out=cT[:, 512:1024],
                                         in0=cT[:, 512:1024],
                                         in1=tmp[:, 512:1024])
                    nc.scalar.activation(out=sgC[:, 0:1024],
                                         in_=S0[:, 0:1024], func=SIG)
                    # tanh(c) for A+B in one call
                    nc.scalar.activation(out=thc[:, 0:1024],
                                         in_=cT[:, 0:1024], func=TANH)
                    # elementwise for C
                    nc.vector.tensor_scalar(
                        out=sgC[:, 768:1024], in0=sgC[:, 768:1024],
                        scalar1=2.0, scalar2=-1.0, op0=MUL, op1=ADD)
                    nc.vector.tensor_mul(out=cT[:, 1024:1280],
                                         in0=cT[:, 1024:1280],
                                         in1=sgC[:, 256:512])
                    nc.gpsimd.tensor_mul(out=tmp[:, 1024:1280],
                                         in0=sgC[:, 0:256],
                                         in1=sgC[:, 768:1024])
                    nc.vector.tensor_add(out=cT[:, 1024:1280],
                                         in0=cT[:, 1024:1280],
                                         in1=tmp[:, 1024:1280])
                    # h = sig(o) * tanh(c)  (DVE: latency-critical)
                    nc.vector.tensor_mul(out=hcur[:, 0:512],
                                         in0=sgA[:, 1024:1536],
                                         in1=thc[:, 0:512])
                    nc.vector.tensor_mul(out=hcur[:, 512:1024],
                                         in0=sgB[:, 1024:1536],
                                         in1=thc[:, 512:1024])
                    nc.scalar.activation(out=thc[:, 1024:1280],
                                         in_=cT[:, 1024:1280], func=TANH)
                    nc.vector.tensor_mul(out=hcur[:, 1024:1280],
                                         in0=sgC[:, 512:768],
                                         in1=thc[:, 1024:1280])
                    if t == 7:
                        ship_slice(0)   # Y cols t=0:7 shipped during step 7

                # Y-proj for the final step, then ship the rest
                hlast = hT1 if (T - 1) % 2 else hT0
                yproj_chunk(T - 1, S0, 1536, 0, 4, hlast)
                yproj_chunk(T - 1, S0, 1600, 512, 4, hlast)
                yproj_chunk(T - 1, S0, 1664, 1024, 2, hlast)
                ship_slice(1)

            # ---- GCN: agg[kt] = sum_sb A[sb,kt].T @ Y[sb], in two column
            # passes: cols 0:128 (t<8, needs only slice 0, overlaps the
            # small slice-1 AllGather + assemble) then cols 128:192. ----
            with tc.tile_pool(name="psC", bufs=1, space="PSUM") as psC, \
                 tc.tile_pool(name="wpC", bufs=1) as wpC:
                for half, (c0, cw) in enumerate(((0, 7 * NF), (7 * NF,
                                                              5 * NF))):
                    hbase = 0 if half == 0 else NSB * 7 * NF
                    for kt in range(NT):
                        pa = psC.tile([P, cw], F32, tag=f"pa{half}", bufs=2)
                        for sb in range(NSB):
                            base = (sb * NT + kt) * P
                            nc.tensor.matmul(
                                out=pa[:],
                                lhsT=asb[:, base:base + P],
                                rhs=ytab[:, hbase + sb * cw:
                                         hbase + (sb + 1) * cw],
                                start=(sb == 0), stop=(sb == NSB - 1))
                        osb = wpC.tile([P, cw], F32, tag=f"osb{half}",
                                       bufs=2)
                        nc.vector.scalar_tensor_tensor(
                            out=osb[:], in0=pa[:],
                            scalar=dinv_sb[:, kt:kt + 1],
                            in1=bout_sb[:, c0:c0 + cw],
                            op0=mybir.AluOpType.mult,
                            op1=mybir.AluOpType.add)
                        nc.sync.dma_start(
                            xhat[kt * P:(kt + 1) * P, c0:c0 + cw], osb[:])

    nc.compile()
    return nc


def _preprocess(z, edge_index, x_mask, W_fc2, b_fc2, W_ih, W_hh, b_ih, b_hh,
                W_gcn, b_gcn, W_fc3, b_fc3):
    z = np.asarray(z, np.float32)
    edge_index = np.asarray(edge_index).astype(np.int64)
    x_mask = np.asarray(x_mask)
    W_fc2 = np.asarray(W_fc2, np.float32)
    b_fc2 = np.asarray(b_fc2, np.float32)
    W_ih = np.asarray(W_ih, np.float32)
    W_hh = np.asarray(W_hh, np.float32)
    b_ih = np.asarray(b_ih, np.float32)
    b_hh = np.asarray(b_hh, np.float32)
    W_gcn = np.asarray(W_gcn, np.float32)
    b_gcn = np.asarray(b_gcn, np.float32)
    W_fc3 = np.asarray(W_fc3, np.float32)
    b_fc3 = np.asarray(b_fc3, np.float32)

    src = edge_index[0]
    dst = edge_index[1]
    deg = np.bincount(dst, minlength=N) + 1.0
    dinv = (1.0 / np.sqrt(deg)).astype(np.float32)
    node_mask = x_mask.any(axis=(1, 2)).astype(np.float32)

    # padded global ids: core = n // 1250, padded = core*1280 + n % 1250
    def pad_id(n):
        return (n // NCN_RAW) * NCN + (n % NCN_RAW)

    src_all = np.concatenate([src, np.arange(N, dtype=np.int64)])
    dst_all = np.concatenate([dst, np.arange(N, dtype=np.int64)])
    psrc = pad_id(src_all)
    pdst = pad_id(dst_all)

    # gate order [i, f, o, g] (pytorch order is i, f, g, o); the g block is
    # scaled by 2 so tanh(g) = 2*sigmoid(2g) - 1 comes out of a plain sigmoid
    perm = np.concatenate([np.arange(0, 128), np.arange(128, 256),
                           np.arange(384, 512), np.arange(256, 384)])
    Wz = (W_ih @ W_fc2.T)[perm]                       # [4H, L]
    btot = (W_ih @ b_fc2 + b_ih + b_hh)[perm]         # [4H]
    wzg_t = np.concatenate([Wz.T, btot[None, :]], axis=0)
    wzg_t[:, 3 * H:] *= 2.0
    wzg_t = wzg_t.astype(np.float16)
    whh_t = W_hh[perm].T.copy()
    whh_t[:, 3 * H:] *= 2.0
    whh_t = np.ascontiguousarray(whh_t.astype(np.float16))
    wfc2_t = np.ascontiguousarray(W_fc2.astype(np.float16))
    b2_t = np.ascontiguousarray(b_fc2.reshape(P, 1))
    Wcomb = np.ascontiguousarray((W_gcn @ W_fc3).astype(np.float16))
    bias = b_gcn @ W_fc3 + b_fc3
    bout_t = np.ascontiguousarray(
        np.tile(bias, (P, T)).astype(np.float32))

    in_maps = []
    acols = NSB * NT * P
    for c in range(NCORES):
        sl = slice(c * NCN_RAW, (c + 1) * NCN_RAW)
        zt_c = np.zeros((L + 1, NCN), np.float16)
        zt_c[:L, :NCN_RAW] = z[sl].T
        zt_c[L, :] = 1.0

        dv_c = np.zeros(NCN, np.float32)
        dv_c[:NCN_RAW] = dinv[sl]
        mk_c = np.zeros(NCN, np.float32)
        mk_c[:NCN_RAW] = node_mask[sl]
        mdv = (dv_c * mk_c).reshape(NT, P)            # [k, p]
        # mdvrep[p, k*NF + f] = mdv[k, p]
        mdvrep_t = np.ascontiguousarray(
            np.repeat(mdv.T[:, :, None], NF, axis=2).reshape(P, NT * NF)
            .astype(np.float32))
        dinv_t = np.ascontiguousarray(dv_c.reshape(NT, P).T)

        m = (pdst // NCN) == c
        s = psrc[m]
        ld = pdst[m] % NCN
        lin = (s % P) * (NSB * NT * P) + ((s // P) * NT + ld // P) * P \
            + (ld % P)
        counts = np.bincount(lin, minlength=P * acols)
        ablk_c = counts.reshape(P, acols).astype(ml_dtypes.float8_e4m3fn)

        in_maps.append({
            "zT": zt_c,
            "wzg": wzg_t,
            "wfc2": wfc2_t,
            "b2": b2_t,
            "whh": whh_t,
            "wcomb": Wcomb,
            "mdvrep": mdvrep_t,
            "dinvt": dinv_t,
            "bout": bout_t,
            "ablk": ablk_c,
        })
    return in_maps


def kernel(z, edge_index, x_mask, W_fc2, b_fc2, W_ih, W_hh, b_ih, b_hh,
           W_gcn, b_gcn, W_fc3, b_fc3):
    global LAST_RESULTS
    in_maps = _preprocess(z, edge_index, x_mask, W_fc2, b_fc2,
                          W_ih, W_hh, b_ih, b_hh,
                          W_gcn, b_gcn, W_fc3, b_fc3)
    if "nc" not in _BUILD_CACHE:
        _BUILD_CACHE["nc"] = _build()
    nc = _BUILD_CACHE["nc"]

    trace = bool(int(os.environ.get("KERNEL_TRACE", "0")))
    res = bass_utils.run_bass_kernel_spmd(
        nc, in_maps, core_ids=list(range(NCORES)), trace=trace)
    LAST_RESULTS = res

    out = np.empty((N, T, NF), np.float32)
    for c in range(NCORES):
        out[c * NCN_RAW:(c + 1) * NCN_RAW] = \
            res.results[c]["xhat"][:NCN_RAW].reshape(NCN_RAW, T, NF)
    return out
